# revision 38
# baseline (speedup 1.0000x reference)
# Swin-style window attention (B=256 windows, N=196, C=768, H=12) on 8 trn2 cores.
# Data-parallel over windows: 32 windows/core. Per core, software-pipelined:
# chunk c's QK/V GEMM units are emission-interleaved with chunk c-1's
# attention units so the in-order PE queue never stalls on exp/DVE deps.
#   qT/kT = Wqk(stationary) @ x.T(bf16, moving)            [o, t] layout
#   V     = x.T(stationary) @ [Wv_h|0]_h (bf16)            [t, 65*h] layout,
#           ones-col per head added via mask (denominator source)
#   per (window, head-pair):
#     S.T  = kT_h.T @ qT_h  (K=64, head-parity row-group packed, concurrent)
#     p    = exp(0.125*S.T) * exp(rpb).T    (2 ACT + one [128,784] mul)
#     OT/cs= [V_h|1].T @ p   (M=65: O.T rows 0:64, denominator row 64)
#     otn  = OT * recip(cs)  (approx-recip on DVE, bcast via sel matmul)
#   yT = Wp(stationary) @ O.T(bf16) + pb  (v_bias pre-folded into pb)
import sys

sys.path.insert(0, "/opt/trn_rl_repo")

from contextlib import ExitStack

import ml_dtypes
import numpy as np

import concourse.bass as bass
import concourse.bacc as bacc
import concourse.mybir as mybir
import concourse.tile as tile
from concourse.bass_utils import run_bass_kernel_spmd

F32 = mybir.dt.float32
F32R = mybir.dt.float32r
BF16 = mybir.dt.bfloat16
AF = mybir.ActivationFunctionType

_NC_CACHE = {}
NCORES = 8
B, N, C, H = 256, 196, 768, 12
HD = C // H  # 64
WPC = B // NCORES  # 32 windows per core
T = WPC * N  # 6272 tokens per core
CB = C // 128  # 6 contraction blocks
CHUNK_W = 2  # windows per chunk
VW = H * (HD + 1)  # V width: per head [V_h | ones-col]
USE_RB_DMA = False  # broadcast recip via stride-0 DMA instead of sel matmul


def _install_ntff_hook():
    """Recreate the antenv.axon_hooks shim so trace=True works under axon."""
    import types

    if "antenv.axon_hooks" in sys.modules:
        return
    mod = types.ModuleType("antenv.axon_hooks")
    mod._hook = None
    mod.set_axon_ntff_profile_hook = lambda h: setattr(mod, "_hook", h)
    mod.get_axon_ntff_profile_hook = lambda: mod._hook
    sys.modules["antenv.axon_hooks"] = mod
    try:
        sys.path.insert(0, "/root/.axon_site/trn_agent_boot")
        from trn_boot import _ntff_profile_via_ctypes

        hook = _ntff_profile_via_ctypes("/opt/axon/libaxon_pjrt.so")
        if hook is not None:
            mod._hook = hook
    except Exception:
        pass


def _build_nc(wpc=WPC, chunk_w=CHUNK_W):
    t_total = wpc * N
    nchunk = wpc // chunk_w
    chunk_t = chunk_w * N

    nc = bacc.Bacc("TRN2", target_bir_lowering=False, debug=False,
                   num_devices=NCORES)
    xT_d = nc.dram_tensor("xT", [C, t_total], BF16, kind="ExternalInput").ap()
    wqk_d = nc.dram_tensor("wqkT", [C, 2 * C], BF16, kind="ExternalInput").ap()
    wv_d = nc.dram_tensor("wvT", [C, VW], BF16, kind="ExternalInput").ap()
    wp_d = nc.dram_tensor("projwT", [C, C], BF16, kind="ExternalInput").ap()
    qb_d = nc.dram_tensor("qbT", [128, CB], F32, kind="ExternalInput").ap()
    pb_d = nc.dram_tensor("pbT", [128, CB], F32, kind="ExternalInput").ap()
    erp_d = nc.dram_tensor("erp2T", [H // 2, 128, 4 * N], BF16,
                           kind="ExternalInput").ap()
    sel_d = nc.dram_tensor("sel", [H // 2, H, 128], BF16,
                           kind="ExternalInput").ap()
    yT_d = nc.dram_tensor("yT", [C, t_total], F32, kind="ExternalOutput").ap()

    tslices = [(i * 512, min(512, chunk_t - i * 512))
               for i in range((chunk_t + 511) // 512)]
    vslices = [(0, 512), (512, VW - 512)]

    with tile.TileContext(nc) as tc, ExitStack() as ctx:
        const = ctx.enter_context(tc.tile_pool(name="const", bufs=1))
        wpool = ctx.enter_context(tc.tile_pool(name="w", bufs=1))
        xpool = ctx.enter_context(tc.tile_pool(name="x", bufs=2))
        qkpool = ctx.enter_context(tc.tile_pool(name="qk", bufs=2))
        vpool = ctx.enter_context(tc.tile_pool(name="v", bufs=2))
        otpool = ctx.enter_context(tc.tile_pool(name="ot", bufs=2))
        ppool = ctx.enter_context(tc.tile_pool(name="p", bufs=4))
        rpool = ctx.enter_context(tc.tile_pool(name="r", bufs=2))
        opool = ctx.enter_context(tc.tile_pool(name="ou", bufs=2))
        ypool = ctx.enter_context(tc.tile_pool(name="y", bufs=2))
        ps_mm = ctx.enter_context(tc.tile_pool(name="psmm", bufs=3,
                                               space="PSUM"))
        ps_st = ctx.enter_context(tc.tile_pool(name="psst", bufs=3,
                                               space="PSUM"))
        ps_ot = ctx.enter_context(tc.tile_pool(name="psot", bufs=2,
                                               space="PSUM"))

        # ---- resident constants / weights ----
        # only qb/pb + wqk + chunk-0 x gate the first matmul; the rest of
        # the weights load while QK runs (deferred unit on other queues)
        qb = const.tile([128, CB], F32)
        nc.sync.dma_start(qb[:], qb_d[:, :])
        pb = const.tile([128, CB], F32)
        nc.sync.dma_start(pb[:], pb_d[:, :])
        wqk, wv, wp = [], [], []
        for cb in range(CB):
            t = wpool.tile([128, 2 * C], BF16, tag=f"wqk{cb}", name="wqk")
            eng = nc.sync if cb % 2 == 0 else nc.scalar
            eng.dma_start(t[:], wqk_d[cb * 128:(cb + 1) * 128, :])
            wqk.append(t)
        vmask = const.tile([128, VW], BF16)
        nc.vector.memset(vmask[:], 0.0)
        for h in range(H):
            nc.vector.memset(vmask[:, h * 65 + 64:h * 65 + 65], 1.0)
        erp2 = []
        sel = []

        def load_weights_late(cx):
            gate = const.tile([1, 16], BF16, name="gate")
            nc.gpsimd.tensor_copy(gate[:], cx["qT"][0][0:1, 0:16])
            for cb in range(CB):
                t = wpool.tile([128, VW], BF16, tag=f"wv{cb}", name="wv")
                nc.scalar.dma_start(t[:], wv_d[cb * 128:(cb + 1) * 128, :])
                wv.append(t)
                t = wpool.tile([128, C], BF16, tag=f"wp{cb}", name="wp")
                nc.gpsimd.dma_start(t[:], wp_d[cb * 128:(cb + 1) * 128, :])
                wp.append(t)
            for hp in range(H // 2):
                t = wpool.tile([128, 4 * N], BF16, tag=f"erp{hp}", name="erp")
                nc.gpsimd.dma_start(t[:], erp_d[hp, :, :])
                erp2.append(t)
            if not USE_RB_DMA:
                for j in range(H // 2):
                    t = const.tile([H, 128], BF16, tag=f"sel{j}",
                                   name="selt")
                    nc.gpsimd.dma_start(t[:], sel_d[j, :, :])
                    sel.append(t)

        # ---------------- unit builders ----------------
        def load_x(cx):
            cx["xt"] = []
            for cb in range(CB):
                t = xpool.tile([128, chunk_t], BF16, tag=f"xt{cb}", name="xt")
                eng = nc.sync if cb % 2 == 0 else nc.scalar
                eng.dma_start(t[:], xT_d[cb * 128:(cb + 1) * 128,
                                         cx["t0"]:cx["t0"] + chunk_t])
                cx["xt"].append(t)

        def gemm_units(cx):
            units = []

            def qk_unit(ob):
                o = ob * 128
                if ob < CB:
                    t = qkpool.tile([128, chunk_t], BF16, tag=f"qT{ob}",
                                    name="qT")
                    cx["qT"][ob] = t
                else:
                    t = qkpool.tile([128, chunk_t + 64], BF16,
                                    tag=f"kT{ob - CB}", name="kT")
                    nc.vector.memset(t[:, chunk_t:chunk_t + 64], 0.0)
                    cx["kT"][ob - CB] = t
                for si, (ts, tl) in enumerate(tslices):
                    pt = ps_mm.tile([128, tl], F32, tag=f"mm{si}", name="pt",
                                    padded_shape=[128, 512])
                    for cb in range(CB):
                        nc.tensor.matmul(
                            pt[:, 0:tl],
                            wqk[cb][:, o:o + 128],
                            cx["xt"][cb][:, ts:ts + tl],
                            start=(cb == 0), stop=(cb == CB - 1))
                    if ob < CB:
                        nc.scalar.activation(t[:, ts:ts + tl], pt[:, 0:tl],
                                             AF.Identity,
                                             bias=qb[:, ob:ob + 1])
                    else:
                        nc.scalar.copy(t[:, ts:ts + tl], pt[:, 0:tl])

            def v_unit(w, moff, mlen):
                trel = w * N + moff
                vt = vpool.tile([128, VW], BF16, tag=f"vb{w}_{moff}",
                                name="vt")
                for si, (noff, nlen) in enumerate(vslices):
                    pv = ps_mm.tile([128, nlen], F32, tag="mm0",
                                    name="pv", padded_shape=[128, 512])
                    for cb in range(CB):
                        nc.tensor.matmul(
                            pv[0:mlen, 0:nlen],
                            cx["xt"][cb][:, trel:trel + mlen],
                            wv[cb][:, noff:noff + nlen],
                            start=(cb == 0), stop=(cb == CB - 1))
                    nc.vector.tensor_add(vt[0:mlen, noff:noff + nlen],
                                         pv[0:mlen, 0:nlen],
                                         vmask[0:mlen, noff:noff + nlen])
                cx["vt"][(w, moff)] = vt

            for ob in range(2 * CB):
                units.append(lambda ob=ob: qk_unit(ob))
            for w in range(chunk_w):
                for (moff, mlen) in ((0, 128), (128, 68)):
                    units.append(
                        lambda w=w, m=moff, ml=mlen: v_unit(w, m, ml))
            return units

        def attn_units(cx):
            units = []

            def a_unit(w, hp):
                h0 = 2 * hp
                wq = w * N
                if "otsb" not in cx:
                    cx["otsb"] = []
                    for ob in range(CB):
                        t = otpool.tile([128, chunk_t], BF16, tag=f"ot{ob}",
                                        name="otsb")
                        cx["otsb"].append(t)
                if w not in cx["otu"]:
                    cx["otu"][w] = opool.tile([65, H * N], F32, tag="otun",
                                              name="otu")
                p = ppool.tile([128, 4 * N], BF16, tag="p", name="p")
                sts = []
                for hi, h in enumerate((h0, h0 + 1)):
                    prt = (h % 2) * 64
                    qh = cx["qT"][hp][prt:prt + 64, wq:wq + N]
                    st = ps_st.tile([128, 2 * N], F32, tag="st", name="st")
                    nc.tensor.matmul(
                        st[:, 0:N],
                        cx["kT"][hp][prt:prt + 64, wq:wq + 128],
                        qh, start=True, stop=True)
                    nc.tensor.matmul(
                        st[:, N:2 * N],
                        cx["kT"][hp][prt:prt + 64, wq + 128:wq + 256],
                        qh, start=True, stop=True)
                    sts.append(st)
                for hi in range(2):
                    nc.scalar.activation(p[:, hi * 2 * N:(hi + 1) * 2 * N],
                                         sts[hi][:], AF.Exp, scale=0.125)
                nc.vector.tensor_mul(p[:], p[:], erp2[hp][:])
                cx["p"][(w, hp)] = p

            def b_unit(w, hp):
                h0 = 2 * hp
                p = cx["p"].pop((w, hp))
                otp = ps_ot.tile([128, 2 * N], F32, tag="ot", name="otp")
                for hi, h in enumerate((h0, h0 + 1)):
                    for bi, (moff, mlen) in enumerate(((0, 128), (128, 68))):
                        nc.tensor.matmul(
                            otp[0:65, hi * N:(hi + 1) * N],
                            cx["vt"][(w, moff)][0:mlen, h * 65:h * 65 + 65],
                            p[0:mlen, hi * 2 * N + bi * N:
                              hi * 2 * N + (bi + 1) * N],
                            start=(bi == 0), stop=(bi == 1))
                nc.vector.tensor_copy(cx["otu"][w][:, h0 * N:(h0 + 2) * N],
                                      otp[0:65, 0:2 * N])

            def wfin_unit(w):
                otu = cx["otu"][w]
                den = rpool.tile([H, N], F32, tag="den", name="den")
                srcrow = otu[64:65, 0:H * N]
                nc.gpsimd.dma_start(
                    den[:], bass.AP(srcrow.tensor, srcrow.offset,
                                    [srcrow.ap[0], [N, H], [1, N]]))
                rec = rpool.tile([H, N], F32, tag="rec", name="rec")
                nc.vector.reciprocal_approx_fast(rec[:], den[:])
                if (w % chunk_w) == 0:
                    cx["recb2"] = rpool.tile([H, chunk_w * N], BF16,
                                             tag="recb", name="recb2")
                wq = (w % chunk_w) * N
                nc.vector.tensor_copy(cx["recb2"][:, wq:wq + N], rec[:])
                if USE_RB_DMA:
                    # fold recip rows into one partition grouped by head
                    # parity (evens then odds), then broadcast each parity
                    # block over 64 partitions via a stride-0 FREE dim
                    rc1 = rpool.tile([1, H * N], BF16, tag="rc1", name="rc1")
                    hw = (H // 2) * N
                    for pi in range(2):
                        src = recb[pi:pi + 1, 0:N]
                        nc.gpsimd.dma_start(
                            bass.AP(rc1.tensor, rc1.offset + pi * hw *
                                    mybir.dt.size(rc1.dtype),
                                    [rc1.ap[0], [N, H // 2], [1, N]]),
                            bass.AP(src.tensor, src.offset,
                                    [[2, H // 2], [1, N]]))
                    rbw = rpool.tile([128, hw], BF16, tag="rbw", name="rbw")
                    for pi, prt in enumerate((0, 64)):
                        src = rc1[0:1, pi * hw:(pi + 1) * hw]
                        nc.gpsimd.dma_start(
                            rbw[prt:prt + 64, :],
                            bass.AP(src.tensor, src.offset,
                                    [[1, 1], [0, 64], [1, hw]]))
                    cx["rbw"][w] = rbw

            def fin_unit(hps):
                # chunk-batched: one sel matmul broadcasts the recips of
                # all windows of the chunk for a head pair
                for hp in hps:
                    h0 = 2 * hp
                    rbp = ps_ot.tile([128, chunk_w * N], F32, tag="ot",
                                     name="rb")
                    nc.tensor.matmul(rbp[:], sel[hp][:], cx["recb2"][:],
                                     start=True, stop=True)
                    for w in range(chunk_w):
                        wq = w * N
                        otu = cx["otu"][w]
                        for hi in range(2):
                            h = h0 + hi
                            prt = (h % 2) * 64
                            nc.vector.tensor_mul(
                                cx["otsb"][hp][prt:prt + 64, wq:wq + N],
                                rbp[prt:prt + 64, wq:wq + N],
                                otu[0:64, h * N:(h + 1) * N])

            pend = []
            for w in range(chunk_w):
                for hp in range(H // 2):
                    units.append(lambda w=w, hp=hp: a_unit(w, hp))
                    if len(pend) >= 2:
                        units.append(pend.pop(0))
                    pend.append(lambda w=w, hp=hp: b_unit(w, hp))
                units.extend(pend)
                pend = []
                units.append(lambda w=w: wfin_unit(w))
            for j in range(3):
                units.append(lambda j=j: fin_unit((2 * j, 2 * j + 1)))
            return units

        def proj_units(cx):
            def proj_unit(opb):
                o = opb * 128
                yt = ypool.tile([128, chunk_t], F32, tag="y", name="yt")
                for si, (ts, tl) in enumerate(tslices):
                    pt = ps_mm.tile([128, tl], F32, tag=f"mm{si}",
                                    name="ppt", padded_shape=[128, 512])
                    for ob in range(CB):
                        nc.tensor.matmul(
                            pt[:, 0:tl],
                            wp[ob][:, o:o + 128],
                            cx["otsb"][ob][:, ts:ts + tl],
                            start=(ob == 0), stop=(ob == CB - 1))
                    nc.scalar.activation(yt[:, ts:ts + tl], pt[:, 0:tl],
                                         AF.Identity,
                                         bias=pb[:, opb:opb + 1])
                nc.sync.dma_start(
                    yT_d[o:o + 128, cx["t0"]:cx["t0"] + chunk_t], yt[:])
            return [lambda opb=opb: proj_unit(opb) for opb in range(CB)]

        # ---------------- pipelined emission ----------------
        def emit_merged(g, a):
            gi = ai = 0
            while gi < len(g) or ai < len(a):
                if ai < len(a) and (gi >= len(g) or
                                    ai * len(g) < gi * len(a)):
                    a[ai]()
                    ai += 1
                else:
                    g[gi]()
                    gi += 1

        prev_attn = None
        pend_proj = []
        for ch in range(nchunk):
            cx = {"t0": ch * chunk_t, "qT": [None] * CB, "kT": [None] * CB,
                  "vt": {}, "otu": {}, "recb": {}, "rbw": {}, "p": {}}
            load_x(cx)
            g = gemm_units(cx)
            if ch == 0:
                g.insert(1, lambda cx=cx: load_weights_late(cx))
            if len(pend_proj) >= 2:
                # proj(ch-2): all deps emitted; rides the dense GEMM stream
                g = g + pend_proj.pop(0)
            emit_merged(g, prev_attn or [])
            prev_attn = attn_units(cx)
            pend_proj.append(proj_units(cx))
        # tail: attn(last) interleaved with proj(last-1), then proj(last)
        emit_merged(pend_proj.pop(0), prev_attn)
        for units in pend_proj:
            for u in units:
                u()

    nc.compile()
    return nc


def _host_prep(x, qkv_w, q_bias, v_bias, rpb_table, proj_w, proj_b, rel_index,
               wpc=WPC):
    x = np.asarray(x, np.float32)
    ncores = x.shape[0] // wpc
    t_total = wpc * N
    xT = np.ascontiguousarray(
        x.reshape(ncores, t_total, C).transpose(0, 2, 1)).astype(
            ml_dtypes.bfloat16)
    qkv_w = np.asarray(qkv_w, np.float32)
    wqkT = np.ascontiguousarray(qkv_w[0:2 * C].T).astype(ml_dtypes.bfloat16)
    wvT_base = qkv_w[2 * C:3 * C].T                           # [C, C]
    wvT = np.zeros((C, VW), np.float32)
    for h in range(H):
        wvT[:, h * 65:h * 65 + 64] = wvT_base[:, h * 64:(h + 1) * 64]
    wvT = np.ascontiguousarray(wvT).astype(ml_dtypes.bfloat16)
    projwT = np.ascontiguousarray(
        np.asarray(proj_w, np.float32).T).astype(ml_dtypes.bfloat16)
    qbT = np.ascontiguousarray(
        np.asarray(q_bias, np.float32).reshape(CB, 128).T)
    # v_bias folds into the proj bias: softmax rows sum to 1, so
    # P@(V+vb) = P@V + vb and y += vb @ proj_w.T
    pb_eff = (np.asarray(proj_b, np.float32) +
              np.asarray(v_bias, np.float32) @
              np.asarray(proj_w, np.float32).T)
    pbT = np.ascontiguousarray(pb_eff.reshape(CB, 128).T)
    rel = np.asarray(rel_index).reshape(N, N)
    rpb = np.asarray(rpb_table, np.float32)[rel]              # [n, m, H]
    erp_full = np.exp(rpb).transpose(2, 1, 0)                 # [H, m, n]
    erpT = np.zeros((H, 128, 2 * N), np.float32)
    erpT[:, :, :N] = erp_full[:, 0:128, :]
    erpT[:, 0:68, N:] = erp_full[:, 128:196, :]
    erp2 = np.ascontiguousarray(
        erpT.reshape(H // 2, 2, 128, 2 * N).transpose(0, 2, 1, 3).reshape(
            H // 2, 128, 4 * N)).astype(ml_dtypes.bfloat16)
    sel = np.zeros((H // 2, H, 128), np.float32)
    for j in range(H // 2):
        sel[j, 2 * j, 0:64] = 1.0
        sel[j, 2 * j + 1, 64:128] = 1.0
    sel = sel.astype(ml_dtypes.bfloat16)
    return xT, wqkT, wvT, projwT, qbT, pbT, erp2, sel


def kernel(x, qkv_w, q_bias, v_bias, rpb_table, proj_w, proj_b, rel_index,
           num_heads=12, _trace=False):
    xT, wqkT, wvT, projwT, qbT, pbT, erp2, sel = _host_prep(
        x, qkv_w, q_bias, v_bias, rpb_table, proj_w, proj_b, rel_index)
    if _trace:
        _install_ntff_hook()
    nc = _NC_CACHE.get("nc")
    if nc is None:
        nc = _build_nc()
        _NC_CACHE["nc"] = nc
    in_maps = [
        {"xT": np.ascontiguousarray(xT[c]), "wqkT": wqkT, "wvT": wvT,
         "projwT": projwT, "qbT": qbT, "pbT": pbT, "erp2T": erp2,
         "sel": sel}
        for c in range(NCORES)
    ]
    res = run_bass_kernel_spmd(nc, in_maps, core_ids=list(range(NCORES)),
                               trace=_trace)
    yT = np.stack([res.results[c]["yT"] for c in range(NCORES)])
    out = np.ascontiguousarray(yT.transpose(0, 2, 1)).reshape(B, N, C)
    if _trace:
        kernel._last_exec_time_ns = res.exec_time_ns
        kernel._last_results = res
    return out.astype(np.float32)
